# revision 1
# baseline (speedup 1.0000x reference)
"""Fused SwiGLU MLP (gate/up/down) Trainium2 Bass kernel.

Problem: y = down( silu(x @ Wg^T) * (x @ Wu^T) ) with
  x  [B=2, S=2048, H=4096]  f32
  Wg [I=11008, H]           f32   (gate proj, [out,in])
  Wu [I=11008, H]           f32
  Wd [H, I]                 f32

Strategy: data-parallel over tokens across the 8 NeuronCores.
Each core gets T = 4096/8 = 512 tokens and the full (replicated) weights,
computing the entire MLP for its token shard.  No collectives; the host
just concatenates the 8 token shards.  Per-core work: 138.6 GFLOP
(compute-bound: ~1.8 ms at the 78.6 TFLOP/s f32 PE roofline) vs ~532 MiB
of HBM traffic (~1.5 ms at ~360 GB/s), overlapped.

Device-side layout decisions (all transposes/tiling done on HOST in numpy
so every device DMA is a plain contiguous partition-major copy):
  x_host  [128, HS=32, T]          x^T tiled: [p, hs, t] = x[t, hs*128+p]
  wg_host [IC=22, 16, 128, 2, 512] Wg^T tiled (I padded 11008->11264)
  wu_host same
  wd_host [IC, 8, 128, 4, 512]     Wd^T tiled
  y out   [TT=4, 128, H]           y[tt*128+p, o]  (natural token-major)

Per-core kernel (per i-chunk ic of 512 padded-I columns):
  gate/up:  psum_g/u[it][128i, T] += Wg^T[h,i].T @ x^T[h,t]   (32 h-subtiles)
  mid:      hm[it] = silu(psum_g) * psum_u          (ACT + DVE)
  down:     psum_y[128t, 512o]    += hm[is][:,tt].T-as-lhsT @ Wd^T[i,o]
            y_sbuf[tt] += psum_y                    (DVE accumulate)
"""

import numpy as np

import concourse.bass as bass
import concourse.mybir as mybir
import concourse.tile as tile
from concourse import bacc
from concourse.bass_utils import run_bass_kernel_spmd

F32 = mybir.dt.float32
F32R = mybir.dt.float32r
P = 128
ICW = 512  # i-chunk width (4 subtiles of 128)
OCW = 512  # o-chunk width

# full-size problem constants
B, S, H, I = 2, 2048, 4096, 11008
NCORES = 8
T = (B * S) // NCORES  # 512 tokens per core
IPAD = 11264           # 22 * 512


def build_nc(T, H, IPAD, wg_bufs=6, wd_bufs=4, hm_bufs=5, sg_bufs=2, mm_dt=F32,
             use_silu=True):
    assert T % P == 0 and T <= 512
    assert H % 512 == 0 and (H // P) % 2 == 0
    assert IPAD % ICW == 0
    HS = H // P       # h subtiles (contraction for gate/up)
    IC = IPAD // ICW  # i chunks
    NO = H // OCW     # o chunks
    TT = T // P       # token tiles

    nc = bacc.Bacc("TRN2", target_bir_lowering=False, debug=False)
    x_d = nc.dram_tensor("x", [P, HS, T], mm_dt, kind="ExternalInput").ap()
    wg_d = nc.dram_tensor("wg", [IC, HS // 2, P, 2, ICW], mm_dt, kind="ExternalInput").ap()
    wu_d = nc.dram_tensor("wu", [IC, HS // 2, P, 2, ICW], mm_dt, kind="ExternalInput").ap()
    wd_d = nc.dram_tensor("wd", [IC, NO, P, ICW // P, OCW], mm_dt, kind="ExternalInput").ap()
    y_d = nc.dram_tensor("y", [TT, P, H], F32, kind="ExternalOutput").ap()

    with tile.TileContext(nc) as tc:
        with (
            tc.tile_pool(name="xp", bufs=1) as xp,
            tc.tile_pool(name="yp", bufs=1) as yp,
            tc.tile_pool(name="wgp", bufs=wg_bufs) as wgp,
            tc.tile_pool(name="wup", bufs=wg_bufs) as wup,
            tc.tile_pool(name="wdp", bufs=wd_bufs) as wdp,
            tc.tile_pool(name="hmp", bufs=hm_bufs) as hmp,
            tc.tile_pool(name="sgp", bufs=sg_bufs) as sgp,
            tc.tile_pool(name="ps", bufs=8, space="PSUM") as ps,
        ):
            # resident x^T (8 MiB) and y accumulator (8 MiB)
            xt = xp.tile([P, HS, T], mm_dt)
            nc.sync.dma_start(out=xt, in_=x_d)
            yt = []
            for tt in range(TT):
                ytile = yp.tile([P, H], F32, name=f"y{tt}", tag=f"y{tt}")
                nc.vector.memset(ytile, 0.0)
                yt.append(ytile)

            for ic in range(IC):
                # ---- gate/up projections, accumulated over all h ----
                psg = [ps.tile([P, T], F32, tag="ps", name=f"psg{k}") for k in range(4)]
                psu = [ps.tile([P, T], F32, tag="ps", name=f"psu{k}") for k in range(4)]
                for j in range(HS // 2):
                    gt = wgp.tile([P, 2, ICW], mm_dt, tag="wg")
                    nc.sync.dma_start(out=gt, in_=wg_d[ic, j])
                    ut = wup.tile([P, 2, ICW], mm_dt, tag="wu")
                    nc.sync.dma_start(out=ut, in_=wu_d[ic, j])
                    for h2 in range(2):
                        hs = 2 * j + h2
                        first, last = hs == 0, hs == HS - 1
                        for it in range(4):
                            nc.tensor.matmul(
                                psg[it],
                                gt[:, h2, it * P:(it + 1) * P],
                                xt[:, hs, :],
                                start=first, stop=last,
                            )
                        for it in range(4):
                            nc.tensor.matmul(
                                psu[it],
                                ut[:, h2, it * P:(it + 1) * P],
                                xt[:, hs, :],
                                start=first, stop=last,
                            )
                # ---- silu(gate) * up -> hm tiles [i128, T] ----
                hms = []
                for it in range(4):
                    sg = sgp.tile([P, T], F32, tag="sg")
                    if use_silu:
                        # native HW silu: one ACT op frees psg immediately
                        nc.scalar.activation(
                            sg, psg[it], mybir.ActivationFunctionType.Silu
                        )
                    else:
                        # CoreSim lacks Silu: sigmoid + extra DVE mul
                        nc.scalar.activation(
                            sg, psg[it], mybir.ActivationFunctionType.Sigmoid
                        )
                        nc.vector.tensor_mul(sg, sg, psg[it])
                    hm = hmp.tile([P, T], mm_dt, tag="hm")
                    nc.vector.tensor_mul(hm, sg, psu[it])
                    hms.append(hm)
                # ---- down projection for this i-chunk ----
                ISUB = ICW // P
                for osc in range(NO):
                    # wd for this (ic, osc) in two half tiles to keep SBUF slim
                    wdts = []
                    for half in range(2):
                        wdt = wdp.tile([P, ISUB // 2, OCW], mm_dt, tag="wd", name=f"wd{half}")
                        nc.sync.dma_start(
                            out=wdt,
                            in_=wd_d[ic, osc, :, half * (ISUB // 2):(half + 1) * (ISUB // 2), :],
                        )
                        wdts.append(wdt)
                    for tt in range(TT):
                        py = ps.tile([P, OCW], F32, tag="ps", name="py")
                        for isub in range(ISUB):
                            nc.tensor.matmul(
                                py,
                                hms[isub][:, tt * P:(tt + 1) * P],
                                wdts[isub // (ISUB // 2)][:, isub % (ISUB // 2), :],
                                start=(isub == 0), stop=(isub == ISUB - 1),
                            )
                        osl = slice(osc * OCW, (osc + 1) * OCW)
                        nc.vector.tensor_add(yt[tt][:, osl], yt[tt][:, osl], py)

            for tt in range(TT):
                nc.sync.dma_start(out=y_d[tt], in_=yt[tt])

    nc.compile()
    return nc


def prep_weights(Wg, Wu, Wd, IPAD):
    """Host-side re-tiling of the weights into the device DMA layouts."""
    Iin, Hh = Wg.shape
    HS = Hh // P
    IC = IPAD // ICW
    NO = Hh // OCW
    f32 = np.float32

    Wg_p = np.zeros((IPAD, Hh), f32)
    Wg_p[:Iin] = Wg
    Wu_p = np.zeros((IPAD, Hh), f32)
    Wu_p[:Iin] = Wu
    Wd_p = np.zeros((Hh, IPAD), f32)
    Wd_p[:, :Iin] = Wd

    # wg[ic, j, p, h2, ii] = Wg_p[ic*ICW + ii, (2j+h2)*128 + p]
    wg_host = np.ascontiguousarray(
        Wg_p.reshape(IC, ICW, HS // 2, 2, P).transpose(0, 2, 4, 3, 1)
    )
    wu_host = np.ascontiguousarray(
        Wu_p.reshape(IC, ICW, HS // 2, 2, P).transpose(0, 2, 4, 3, 1)
    )
    # wd[ic, osc, p, isub, oo] = Wd_p[osc*OCW + oo, ic*ICW + isub*128 + p]
    wd_host = np.ascontiguousarray(
        Wd_p.reshape(NO, OCW, IC, ICW // P, P).transpose(2, 0, 4, 3, 1)
    )
    return wg_host, wu_host, wd_host


def prep_x_shard(x2, c, T):
    """x2 [tokens, H] -> core c's [128, HS, T] tile layout."""
    Hh = x2.shape[1]
    xs = x2[c * T:(c + 1) * T]  # [T, H]
    return np.ascontiguousarray(xs.reshape(T, Hh // P, P).transpose(2, 1, 0))


def run_on_cores(nc, in_maps, **kwargs):
    return run_bass_kernel_spmd(nc, in_maps, core_ids=list(range(len(in_maps))), **kwargs)


_NC_CACHE = {}

# matmul dtype mode: "f32" (exact, 4 PE cycles/row) or "f32r" (tf32-like,
# 1 PE cycle/row, ~2e-4 rel err)
MM_MODE = "f32r"


def _get_nc(mode=None):
    mode = mode or MM_MODE
    key = (T, H, IPAD, mode)
    if key not in _NC_CACHE:
        _NC_CACHE[key] = build_nc(T, H, IPAD, mm_dt=(F32R if mode == "f32r" else F32))
    return _NC_CACHE[key]


def kernel(x, Wg, Wu, Wd, _trace=False, _trace_kwargs=None, _mode=None):
    x = np.asarray(x, np.float32)
    Wg = np.asarray(Wg, np.float32)
    Wu = np.asarray(Wu, np.float32)
    Wd = np.asarray(Wd, np.float32)

    nc = _get_nc(_mode)
    wg_host, wu_host, wd_host = prep_weights(Wg, Wu, Wd, IPAD)
    x2 = x.reshape(B * S, H)
    in_maps = [
        {
            "x": prep_x_shard(x2, c, T),
            "wg": wg_host,
            "wu": wu_host,
            "wd": wd_host,
        }
        for c in range(NCORES)
    ]
    kwargs = {}
    if _trace:
        kwargs["trace"] = True
        kwargs.update(_trace_kwargs or {})
    res = run_on_cores(nc, in_maps, **kwargs)
    shards = [res.results[c]["y"].reshape(T, H) for c in range(NCORES)]
    y = np.concatenate(shards, axis=0).reshape(B, S, H)
    if _trace:
        return y, res
    return y



# revision 4
# speedup vs baseline: 1.1771x; 1.1771x over previous
"""Fused SwiGLU MLP (gate/up/down) Trainium2 Bass kernel.

Problem: y = down( silu(x @ Wg^T) * (x @ Wu^T) ) with
  x  [B=2, S=2048, H=4096]  f32
  Wg [I=11008, H]           f32   (gate proj, [out,in])
  Wu [I=11008, H]           f32
  Wd [H, I]                 f32

Strategy: data-parallel over tokens across the 8 NeuronCores.
Each core gets T = 4096/8 = 512 tokens and the full (replicated) weights,
computing the entire MLP for its token shard.  No collectives; the host
just concatenates the 8 token shards.

Per-core work: 138.6 GFLOP -> 8256 matmuls of [128k x 128m] @ [128 x 512]
= 4.23M PE cycles = 1.76 ms at the 2.4 GHz / 1-cycle-per-row bf16 rate.
The matmul path runs in bf16 (weights, x, hm), which (a) halves HBM
traffic vs f32 to ~286 MB/core (~0.95 ms at ~300 GB/s, fully hidden
under compute -- the f32 version was DMA-bound at 2.14 ms), and
(b) enables fast-weight-load so LDWEIGHTS hides under the matmuls.
PSUM accumulation and the y accumulator stay f32.

I=11008 is processed exactly (no padding) as 21 i-chunks of 512 plus one
tail chunk of 256, grouped in PAIRS: gate/up for chunk A, gate/up for
chunk B, then one combined down pass whose PSUM groups accumulate over
both chunks' hm (8-12 matmuls per PSUM bank before the single DVE
y-accumulate).  Pairing keeps the PE ahead of the DVE on PSUM-bank
recycling (a 2-matmul tail group is faster than its 0.7us DVE drain),
halves the y-accumulate traffic, and halves the chunk-boundary count.

Device-side layouts (all transposes/tiling done on HOST in numpy so
every device DMA is a plain contiguous partition-major copy):
  x_host  [128, HS=32, T]          x^T tiled: [p, hs, t] = x[t, hs*128+p]
  wg main [21, 16, 128, 2, 512]    Wg^T tiled; tail [16, 128, 2, 256]
  wu      same
  wd main [21, 8, 128, 4, 512]     Wd^T tiled; tail [8, 128, 2, 512]
  y out   [TT=4, 128, H]           y[tt*128+p, o]  (natural token-major)

Per-core kernel (per i-chunk of W columns, nsub = W/128):
  gate/up:  psum_g/u[it][128i, T] += Wg^T[h,i].T @ x^T[h,t]   (32 h-subtiles,
            g/u interleaved so ACT+DVE drain pipelines into the last MMs)
  mid:      hm[it] = silu(psum_g) * psum_u          (ACT + DVE, hm in bf16)
  down:     psum_y[128t, 512o]    += hm[is][:,tt].T-as-lhsT @ Wd^T[i,o]
            y_sbuf[tt] += psum_y                    (DVE accumulate, f32)
"""

import numpy as np
import ml_dtypes

import concourse.bass as bass
import concourse.mybir as mybir
import concourse.tile as tile
from concourse import bacc
from concourse.bass_utils import run_bass_kernel_spmd

F32 = mybir.dt.float32
BF16 = mybir.dt.bfloat16
P = 128
ICW = 512  # main i-chunk width (4 subtiles of 128)
OCW = 512  # o-chunk width

# full-size problem constants
B, S, H, I = 2, 2048, 4096, 11008
NCORES = 8
T = (B * S) // NCORES  # 512 tokens per core

NP_BF16 = ml_dtypes.bfloat16


def make_chunks(Iin):
    """Split Iin into chunks of ICW plus at most one smaller tail chunk."""
    assert Iin % P == 0
    n_main = Iin // ICW
    chunks = [ICW] * n_main
    if Iin % ICW:
        chunks.append(Iin % ICW)
    return chunks


def build_nc(T, H, Iin, wg_bufs=10, wd_bufs=6, hm_bufs=12, sg_bufs=4, mm_dt=BF16,
             use_silu=True):
    assert T % P == 0 and T <= 512
    assert H % 512 == 0 and (H // P) % 2 == 0
    HS = H // P       # h subtiles (contraction for gate/up)
    NO = H // OCW     # o chunks
    TT = T // P       # token tiles
    chunks = make_chunks(Iin)
    n_main = sum(1 for w in chunks if w == ICW)
    w_tail = chunks[-1] if chunks[-1] != ICW else 0
    # pair up chunks for the combined down pass
    pairs = [list(range(i, min(i + 2, len(chunks)))) for i in range(0, len(chunks), 2)]

    nc = bacc.Bacc("TRN2", target_bir_lowering=False, debug=False)
    x_d = nc.dram_tensor("x", [P, HS, T], mm_dt, kind="ExternalInput").ap()
    wg_d = nc.dram_tensor("wg", [n_main, HS // 2, P, 2, ICW], mm_dt, kind="ExternalInput").ap()
    wu_d = nc.dram_tensor("wu", [n_main, HS // 2, P, 2, ICW], mm_dt, kind="ExternalInput").ap()
    wd_d = nc.dram_tensor("wd", [n_main, NO, P, ICW // P, OCW], mm_dt, kind="ExternalInput").ap()
    if w_tail:
        wgt_d = nc.dram_tensor("wgt", [HS // 2, P, 2, w_tail], mm_dt, kind="ExternalInput").ap()
        wut_d = nc.dram_tensor("wut", [HS // 2, P, 2, w_tail], mm_dt, kind="ExternalInput").ap()
        wdt_d = nc.dram_tensor("wdt", [NO, P, w_tail // P, OCW], mm_dt, kind="ExternalInput").ap()
    y_d = nc.dram_tensor("y", [TT, P, H], F32, kind="ExternalOutput").ap()

    with tile.TileContext(nc) as tc:
        with (
            tc.tile_pool(name="xp", bufs=1) as xp,
            tc.tile_pool(name="yp", bufs=1) as yp,
            tc.tile_pool(name="wgp", bufs=wg_bufs) as wgp,
            tc.tile_pool(name="wup", bufs=wg_bufs) as wup,
            tc.tile_pool(name="wdp", bufs=wd_bufs) as wdp,
            tc.tile_pool(name="hmp", bufs=hm_bufs) as hmp,
            tc.tile_pool(name="sgp", bufs=sg_bufs) as sgp,
            tc.tile_pool(name="ps", bufs=8, space="PSUM") as ps,
        ):
            # resident x^T (4 MiB bf16), loaded smallest-slice-first so the
            # first matmuls don't wait on the whole transfer
            xt = xp.tile([P, HS, T], mm_dt)
            xoff, xw = 0, 2
            while xoff < HS:
                w = min(xw, HS - xoff)
                nc.sync.dma_start(
                    out=xt[:, xoff:xoff + w, :], in_=x_d[:, xoff:xoff + w, :]
                )
                xoff += w
                xw *= 2
            # resident y accumulator (8 MiB f32); the first down pass writes
            # it with tensor_copy so no memset is needed
            yt = [yp.tile([P, H], F32, name=f"y{tt}", tag=f"y{tt}") for tt in range(TT)]

            for ip, pair in enumerate(pairs):
                first_pair = ip == 0
                last_pair = ip == len(pairs) - 1
                hm_all = []  # [(hms, ic)] for this pair
                for ic in pair:
                    W = chunks[ic]
                    nsub = W // P
                    is_tail = w_tail and ic == len(chunks) - 1
                    # ---- gate/up projections, accumulated over all h ----
                    psg = [ps.tile([P, T], F32, tag="ps", name=f"psg{k}") for k in range(nsub)]
                    psu = [ps.tile([P, T], F32, tag="ps", name=f"psu{k}") for k in range(nsub)]
                    for j in range(HS // 2):
                        gt = wgp.tile([P, 2, W], mm_dt, tag="wg")
                        nc.sync.dma_start(out=gt, in_=wgt_d[j] if is_tail else wg_d[ic, j])
                        ut = wup.tile([P, 2, W], mm_dt, tag="wu")
                        nc.sync.dma_start(out=ut, in_=wut_d[j] if is_tail else wu_d[ic, j])
                        for h2 in range(2):
                            hs = 2 * j + h2
                            first, last = hs == 0, hs == HS - 1
                            for it in range(nsub):
                                nc.tensor.matmul(
                                    psg[it],
                                    gt[:, h2, it * P:(it + 1) * P],
                                    xt[:, hs, :],
                                    start=first, stop=last,
                                )
                                nc.tensor.matmul(
                                    psu[it],
                                    ut[:, h2, it * P:(it + 1) * P],
                                    xt[:, hs, :],
                                    start=first, stop=last,
                                )
                    # ---- silu(gate) * up -> hm tiles [i128, T] in bf16 ----
                    hms = []
                    for it in range(nsub):
                        sg = sgp.tile([P, T], F32, tag="sg")
                        if use_silu:
                            # native HW silu: one ACT op frees psg immediately
                            nc.scalar.activation(
                                sg, psg[it], mybir.ActivationFunctionType.Silu
                            )
                        else:
                            # CoreSim lacks Silu: sigmoid + extra DVE mul
                            nc.scalar.activation(
                                sg, psg[it], mybir.ActivationFunctionType.Sigmoid
                            )
                            nc.vector.tensor_mul(sg, sg, psg[it])
                        hm = hmp.tile([P, T], mm_dt, tag="hm")
                        nc.vector.tensor_mul(hm, sg, psu[it])
                        hms.append(hm)
                    hm_all.append((hms, ic))
                # ---- combined down projection for this pair of i-chunks ----
                for osc in range(NO):
                    mm_ops = []  # (hm tile, wd lhsT slice) per 128-wide i-subtile
                    for hms, ic in hm_all:
                        nsub = len(hms)
                        is_tail = w_tail and ic == len(chunks) - 1
                        wdt = wdp.tile([P, nsub, OCW], mm_dt, tag="wd")
                        nc.sync.dma_start(out=wdt, in_=wdt_d[osc] if is_tail else wd_d[ic, osc])
                        for s in range(nsub):
                            mm_ops.append((hms[s], wdt[:, s, :]))
                    osl = slice(osc * OCW, (osc + 1) * OCW)
                    for tt in range(TT):
                        py = ps.tile([P, OCW], F32, tag="ps", name="py")
                        for q, (hm, wsl) in enumerate(mm_ops):
                            nc.tensor.matmul(
                                py,
                                hm[:, tt * P:(tt + 1) * P],
                                wsl,
                                start=(q == 0), stop=(q == len(mm_ops) - 1),
                            )
                        if first_pair:
                            nc.vector.tensor_copy(yt[tt][:, osl], py)
                        else:
                            nc.vector.tensor_add(yt[tt][:, osl], yt[tt][:, osl], py)
                        if last_pair:
                            # y[:, osl] is final -- stream it out now
                            nc.sync.dma_start(out=y_d[tt][:, osl], in_=yt[tt][:, osl])

    nc.compile()
    return nc


def prep_weights(Wg, Wu, Wd, Iin):
    """Host-side re-tiling of the weights into the device DMA layouts."""
    _, Hh = Wg.shape
    HS = Hh // P
    NO = Hh // OCW
    chunks = make_chunks(Iin)
    n_main = sum(1 for w in chunks if w == ICW)
    w_tail = chunks[-1] if chunks[-1] != ICW else 0
    Im = n_main * ICW

    def gu_tile(Wx, off, w):
        # [j, p, h2, ii] = Wx[off + ii, (2j+h2)*128 + p]
        return Wx[off:off + w].reshape(w, HS // 2, 2, P).transpose(1, 3, 2, 0)

    def d_tile(Wx, off, w):
        # [osc, p, isub, oo] = Wx[osc*OCW + oo, off + isub*128 + p]
        return Wx[:, off:off + w].reshape(NO, OCW, w // P, P).transpose(0, 3, 2, 1)

    out = {
        "wg": np.ascontiguousarray(
            Wg[:Im].reshape(n_main, ICW, HS // 2, 2, P).transpose(0, 2, 4, 3, 1)
        ).astype(NP_BF16),
        "wu": np.ascontiguousarray(
            Wu[:Im].reshape(n_main, ICW, HS // 2, 2, P).transpose(0, 2, 4, 3, 1)
        ).astype(NP_BF16),
        "wd": np.ascontiguousarray(
            Wd[:, :Im].reshape(NO, OCW, n_main, ICW // P, P).transpose(2, 0, 4, 3, 1)
        ).astype(NP_BF16),
    }
    if w_tail:
        out["wgt"] = np.ascontiguousarray(gu_tile(Wg, Im, w_tail)).astype(NP_BF16)
        out["wut"] = np.ascontiguousarray(gu_tile(Wu, Im, w_tail)).astype(NP_BF16)
        out["wdt"] = np.ascontiguousarray(d_tile(Wd, Im, w_tail)).astype(NP_BF16)
    return out


def prep_x_shard(x2, c, T):
    """x2 [tokens, H] -> core c's [128, HS, T] tile layout (bf16)."""
    Hh = x2.shape[1]
    xs = x2[c * T:(c + 1) * T]  # [T, H]
    return np.ascontiguousarray(
        xs.reshape(T, Hh // P, P).transpose(2, 1, 0)
    ).astype(NP_BF16)


def run_on_cores(nc, in_maps, **kwargs):
    return run_bass_kernel_spmd(nc, in_maps, core_ids=list(range(len(in_maps))), **kwargs)


_NC_CACHE = {}


def _get_nc():
    key = (T, H, I)
    if key not in _NC_CACHE:
        _NC_CACHE[key] = build_nc(T, H, I)
    return _NC_CACHE[key]


def kernel(x, Wg, Wu, Wd, _trace=False, _trace_kwargs=None):
    x = np.asarray(x, np.float32)
    Wg = np.asarray(Wg, np.float32)
    Wu = np.asarray(Wu, np.float32)
    Wd = np.asarray(Wd, np.float32)

    nc = _get_nc()
    w_maps = prep_weights(Wg, Wu, Wd, I)
    x2 = x.reshape(B * S, H)
    in_maps = [{"x": prep_x_shard(x2, c, T), **w_maps} for c in range(NCORES)]
    kwargs = {}
    if _trace:
        kwargs["trace"] = True
        kwargs.update(_trace_kwargs or {})
    res = run_on_cores(nc, in_maps, **kwargs)
    shards = [res.results[c]["y"].reshape(T, H) for c in range(NCORES)]
    y = np.concatenate(shards, axis=0).reshape(B, S, H)
    if _trace:
        return y, res
    return y


# revision 7
# speedup vs baseline: 1.1783x; 1.0010x over previous
"""Fused SwiGLU MLP (gate/up/down) Trainium2 Bass kernel.

Problem: y = down( silu(x @ Wg^T) * (x @ Wu^T) ) with
  x  [B=2, S=2048, H=4096]  f32
  Wg [I=11008, H]           f32   (gate proj, [out,in])
  Wu [I=11008, H]           f32
  Wd [H, I]                 f32

Strategy: data-parallel over tokens across the 8 NeuronCores.
Each core gets T = 4096/8 = 512 tokens and the full (replicated) weights,
computing the entire MLP for its token shard.  No collectives; the host
just concatenates the 8 token shards.

Per-core work: 138.6 GFLOP -> 8256 matmuls of [128k x 128m] @ [128 x 512]
= 4.23M PE cycles = 1.76 ms at the 2.4 GHz / 1-cycle-per-row bf16 rate.
The matmul path runs in bf16 (weights, x, hm), which (a) halves HBM
traffic vs f32 to ~286 MB/core (~0.95 ms at ~300 GB/s, fully hidden
under compute -- the f32 version was DMA-bound at 2.14 ms), and
(b) enables fast-weight-load so LDWEIGHTS hides under the matmuls.
PSUM accumulation and the y accumulator stay f32.

I=11008 is processed exactly (no padding) as 21 i-chunks of 512 plus one
tail chunk of 256, grouped in PAIRS: gate/up for chunk A, gate/up for
chunk B, then one combined down pass whose PSUM groups accumulate over
both chunks' hm (8-12 matmuls per PSUM bank before the single DVE
y-accumulate).  Pairing keeps the PE ahead of the DVE on PSUM-bank
recycling (a 2-matmul tail group is faster than its 0.7us DVE drain),
halves the y-accumulate traffic, and halves the chunk-boundary count.

Device-side layouts (all transposes/tiling done on HOST in numpy so
every device DMA is a plain contiguous partition-major copy):
  x_host  [128, HS=32, T]          x^T tiled: [p, hs, t] = x[t, hs*128+p]
  wg main [21, 16, 128, 2, 512]    Wg^T tiled; tail [16, 128, 2, 256]
  wu      same
  wd main [21, 8, 128, 4, 512]     Wd^T tiled; tail [8, 128, 2, 512]
  y out   [TT=4, 128, H]           y[tt*128+p, o]  (natural token-major)

Per-core kernel (per i-chunk of W columns, nsub = W/128):
  gate/up:  psum_g/u[it][128i, T] += Wg^T[h,i].T @ x^T[h,t]   (32 h-subtiles,
            g/u interleaved so ACT+DVE drain pipelines into the last MMs)
  mid:      hm[it] = silu(psum_g) * psum_u          (ACT + DVE, hm in bf16)
  down:     psum_y[128t, 512o]    += hm[is][:,tt].T-as-lhsT @ Wd^T[i,o]
            y_sbuf[tt] += psum_y                    (DVE accumulate, f32)
"""

import numpy as np
import ml_dtypes

import concourse.bass as bass
import concourse.mybir as mybir
import concourse.tile as tile
from concourse import bacc
from concourse.bass_utils import run_bass_kernel_spmd

F32 = mybir.dt.float32
BF16 = mybir.dt.bfloat16
P = 128
ICW = 512  # main i-chunk width (4 subtiles of 128)
OCW = 512  # o-chunk width

# full-size problem constants
B, S, H, I = 2, 2048, 4096, 11008
NCORES = 8
T = (B * S) // NCORES  # 512 tokens per core

NP_BF16 = ml_dtypes.bfloat16


def make_chunks(Iin):
    """Split Iin into chunks of ICW plus at most one smaller tail chunk."""
    assert Iin % P == 0
    n_main = Iin // ICW
    chunks = [ICW] * n_main
    if Iin % ICW:
        chunks.append(Iin % ICW)
    return chunks


def build_nc(T, H, Iin, wg_bufs=10, wd_bufs=6, hm_bufs=12, sg_bufs=4, mm_dt=BF16,
             use_silu=True):
    assert T % P == 0 and T <= 512
    assert H % 512 == 0 and (H // P) % 2 == 0
    HS = H // P       # h subtiles (contraction for gate/up)
    NO = H // OCW     # o chunks
    TT = T // P       # token tiles
    chunks = make_chunks(Iin)
    n_main = sum(1 for w in chunks if w == ICW)
    w_tail = chunks[-1] if chunks[-1] != ICW else 0
    # pair up chunks for the combined down pass
    pairs = [list(range(i, min(i + 2, len(chunks)))) for i in range(0, len(chunks), 2)]

    nc = bacc.Bacc("TRN2", target_bir_lowering=False, debug=False)
    x_d = nc.dram_tensor("x", [P, HS, T], mm_dt, kind="ExternalInput").ap()
    wg_d = nc.dram_tensor("wg", [n_main, HS // 2, P, 2, ICW], mm_dt, kind="ExternalInput").ap()
    wu_d = nc.dram_tensor("wu", [n_main, HS // 2, P, 2, ICW], mm_dt, kind="ExternalInput").ap()
    wd_d = nc.dram_tensor("wd", [n_main, NO, P, ICW // P, OCW], mm_dt, kind="ExternalInput").ap()
    if w_tail:
        wgt_d = nc.dram_tensor("wgt", [HS // 2, P, 2, w_tail], mm_dt, kind="ExternalInput").ap()
        wut_d = nc.dram_tensor("wut", [HS // 2, P, 2, w_tail], mm_dt, kind="ExternalInput").ap()
        wdt_d = nc.dram_tensor("wdt", [NO, P, w_tail // P, OCW], mm_dt, kind="ExternalInput").ap()
    y_d = nc.dram_tensor("y", [TT, P, H], F32, kind="ExternalOutput").ap()

    with tile.TileContext(nc) as tc:
        with (
            tc.tile_pool(name="xp", bufs=1) as xp,
            tc.tile_pool(name="yp", bufs=1) as yp,
            tc.tile_pool(name="wgp", bufs=wg_bufs) as wgp,
            tc.tile_pool(name="wup", bufs=wg_bufs) as wup,
            tc.tile_pool(name="wdp", bufs=wd_bufs) as wdp,
            tc.tile_pool(name="hmp", bufs=hm_bufs) as hmp,
            tc.tile_pool(name="sgp", bufs=sg_bufs) as sgp,
            tc.tile_pool(name="ps", bufs=8, space="PSUM") as ps,
        ):
            # resident x^T (4 MiB bf16) in 4 independent tiles so the first
            # matmuls only wait on the first quarter; the first chunk's j=0
            # weight tiles are issued right after x quarter 0
            hs_per = max(HS // 4, 1)
            xts = [xp.tile([P, hs_per, T], mm_dt, tag=f"x{q}", name=f"x{q}")
                   for q in range(HS // hs_per)]
            nc.sync.dma_start(out=xts[0], in_=x_d[:, 0:hs_per, :])
            pre_gt = wgp.tile([P, 2, chunks[0]], mm_dt, tag="wg")
            nc.sync.dma_start(
                out=pre_gt, in_=wgt_d[0] if (w_tail and len(chunks) == 1) else wg_d[0, 0])
            pre_ut = wup.tile([P, 2, chunks[0]], mm_dt, tag="wu")
            nc.sync.dma_start(
                out=pre_ut, in_=wut_d[0] if (w_tail and len(chunks) == 1) else wu_d[0, 0])
            for q in range(1, len(xts)):
                nc.sync.dma_start(
                    out=xts[q], in_=x_d[:, q * hs_per:(q + 1) * hs_per, :])

            def xsl(hs):
                return xts[hs // hs_per][:, hs % hs_per, :]
            # resident y accumulator (8 MiB f32); the first down pass writes
            # it with tensor_copy so no memset is needed
            yt = [yp.tile([P, H], F32, name=f"y{tt}", tag=f"y{tt}") for tt in range(TT)]

            for ip, pair in enumerate(pairs):
                first_pair = ip == 0
                last_pair = ip == len(pairs) - 1
                hm_all = []  # [(hms, ic)] for this pair
                for ic in pair:
                    W = chunks[ic]
                    nsub = W // P
                    is_tail = w_tail and ic == len(chunks) - 1
                    # ---- gate/up projections, accumulated over all h ----
                    psg = [ps.tile([P, T], F32, tag="ps", name=f"psg{k}") for k in range(nsub)]
                    psu = [ps.tile([P, T], F32, tag="ps", name=f"psu{k}") for k in range(nsub)]
                    for j in range(HS // 2):
                        if ic == 0 and j == 0:
                            gt, ut = pre_gt, pre_ut
                        else:
                            gt = wgp.tile([P, 2, W], mm_dt, tag="wg")
                            nc.sync.dma_start(out=gt, in_=wgt_d[j] if is_tail else wg_d[ic, j])
                            ut = wup.tile([P, 2, W], mm_dt, tag="wu")
                            nc.sync.dma_start(out=ut, in_=wut_d[j] if is_tail else wu_d[ic, j])
                        for h2 in range(2):
                            hs = 2 * j + h2
                            first, last = hs == 0, hs == HS - 1
                            for it in range(nsub):
                                nc.tensor.matmul(
                                    psg[it],
                                    gt[:, h2, it * P:(it + 1) * P],
                                    xsl(hs),
                                    start=first, stop=last,
                                )
                                nc.tensor.matmul(
                                    psu[it],
                                    ut[:, h2, it * P:(it + 1) * P],
                                    xsl(hs),
                                    start=first, stop=last,
                                )
                    # ---- silu(gate) * up -> hm tiles [i128, T] in bf16 ----
                    hms = []
                    for it in range(nsub):
                        sg = sgp.tile([P, T], F32, tag="sg")
                        if use_silu:
                            # native HW silu: one ACT op frees psg immediately
                            nc.scalar.activation(
                                sg, psg[it], mybir.ActivationFunctionType.Silu
                            )
                        else:
                            # CoreSim lacks Silu: sigmoid + extra DVE mul
                            nc.scalar.activation(
                                sg, psg[it], mybir.ActivationFunctionType.Sigmoid
                            )
                            nc.vector.tensor_mul(sg, sg, psg[it])
                        hm = hmp.tile([P, T], mm_dt, tag="hm")
                        nc.vector.tensor_mul(hm, sg, psu[it])
                        hms.append(hm)
                    hm_all.append((hms, ic))
                # ---- combined down projection for this pair of i-chunks ----
                for osc in range(NO):
                    mm_ops = []  # (hm tile, wd lhsT slice) per 128-wide i-subtile
                    for hms, ic in hm_all:
                        nsub = len(hms)
                        is_tail = w_tail and ic == len(chunks) - 1
                        wdt = wdp.tile([P, nsub, OCW], mm_dt, tag="wd")
                        nc.sync.dma_start(out=wdt, in_=wdt_d[osc] if is_tail else wd_d[ic, osc])
                        for s in range(nsub):
                            mm_ops.append((hms[s], wdt[:, s, :]))
                    osl = slice(osc * OCW, (osc + 1) * OCW)
                    for tt in range(TT):
                        py = ps.tile([P, OCW], F32, tag="ps", name="py")
                        for q, (hm, wsl) in enumerate(mm_ops):
                            nc.tensor.matmul(
                                py,
                                hm[:, tt * P:(tt + 1) * P],
                                wsl,
                                start=(q == 0), stop=(q == len(mm_ops) - 1),
                            )
                        if first_pair:
                            nc.vector.tensor_copy(yt[tt][:, osl], py)
                        else:
                            nc.vector.tensor_add(yt[tt][:, osl], yt[tt][:, osl], py)
                        if last_pair:
                            # y[:, osl] is final -- stream it out now
                            nc.sync.dma_start(out=y_d[tt][:, osl], in_=yt[tt][:, osl])

    nc.compile()
    return nc


def prep_weights(Wg, Wu, Wd, Iin):
    """Host-side re-tiling of the weights into the device DMA layouts."""
    _, Hh = Wg.shape
    HS = Hh // P
    NO = Hh // OCW
    chunks = make_chunks(Iin)
    n_main = sum(1 for w in chunks if w == ICW)
    w_tail = chunks[-1] if chunks[-1] != ICW else 0
    Im = n_main * ICW

    def gu_tile(Wx, off, w):
        # [j, p, h2, ii] = Wx[off + ii, (2j+h2)*128 + p]
        return Wx[off:off + w].reshape(w, HS // 2, 2, P).transpose(1, 3, 2, 0)

    def d_tile(Wx, off, w):
        # [osc, p, isub, oo] = Wx[osc*OCW + oo, off + isub*128 + p]
        return Wx[:, off:off + w].reshape(NO, OCW, w // P, P).transpose(0, 3, 2, 1)

    out = {
        "wg": np.ascontiguousarray(
            Wg[:Im].reshape(n_main, ICW, HS // 2, 2, P).transpose(0, 2, 4, 3, 1)
        ).astype(NP_BF16),
        "wu": np.ascontiguousarray(
            Wu[:Im].reshape(n_main, ICW, HS // 2, 2, P).transpose(0, 2, 4, 3, 1)
        ).astype(NP_BF16),
        "wd": np.ascontiguousarray(
            Wd[:, :Im].reshape(NO, OCW, n_main, ICW // P, P).transpose(2, 0, 4, 3, 1)
        ).astype(NP_BF16),
    }
    if w_tail:
        out["wgt"] = np.ascontiguousarray(gu_tile(Wg, Im, w_tail)).astype(NP_BF16)
        out["wut"] = np.ascontiguousarray(gu_tile(Wu, Im, w_tail)).astype(NP_BF16)
        out["wdt"] = np.ascontiguousarray(d_tile(Wd, Im, w_tail)).astype(NP_BF16)
    return out


def prep_x_shard(x2, c, T):
    """x2 [tokens, H] -> core c's [128, HS, T] tile layout (bf16)."""
    Hh = x2.shape[1]
    xs = x2[c * T:(c + 1) * T]  # [T, H]
    return np.ascontiguousarray(
        xs.reshape(T, Hh // P, P).transpose(2, 1, 0)
    ).astype(NP_BF16)


def run_on_cores(nc, in_maps, **kwargs):
    return run_bass_kernel_spmd(nc, in_maps, core_ids=list(range(len(in_maps))), **kwargs)


_NC_CACHE = {}


def _get_nc():
    key = (T, H, I)
    if key not in _NC_CACHE:
        _NC_CACHE[key] = build_nc(T, H, I)
    return _NC_CACHE[key]


def kernel(x, Wg, Wu, Wd, _trace=False, _trace_kwargs=None):
    x = np.asarray(x, np.float32)
    Wg = np.asarray(Wg, np.float32)
    Wu = np.asarray(Wu, np.float32)
    Wd = np.asarray(Wd, np.float32)

    nc = _get_nc()
    w_maps = prep_weights(Wg, Wu, Wd, I)
    x2 = x.reshape(B * S, H)
    in_maps = [{"x": prep_x_shard(x2, c, T), **w_maps} for c in range(NCORES)]
    kwargs = {}
    if _trace:
        kwargs["trace"] = True
        kwargs.update(_trace_kwargs or {})
    res = run_on_cores(nc, in_maps, **kwargs)
    shards = [res.results[c]["y"].reshape(T, H) for c in range(NCORES)]
    y = np.concatenate(shards, axis=0).reshape(B, S, H)
    if _trace:
        return y, res
    return y


# revision 10
# speedup vs baseline: 1.1836x; 1.0045x over previous
"""Fused SwiGLU MLP (gate/up/down) Trainium2 Bass kernel.

Problem: y = down( silu(x @ Wg^T) * (x @ Wu^T) ) with
  x  [B=2, S=2048, H=4096]  f32
  Wg [I=11008, H]           f32   (gate proj, [out,in])
  Wu [I=11008, H]           f32
  Wd [H, I]                 f32

Strategy: data-parallel over tokens across the 8 NeuronCores.
Each core gets T = 4096/8 = 512 tokens and the full (replicated) weights,
computing the entire MLP for its token shard.  No collectives; the host
just concatenates the 8 token shards.

Per-core work: 138.6 GFLOP -> 8256 matmuls of [128k x 128m] @ [128 x 512]
= 4.23M PE cycles = 1.76 ms at the 2.4 GHz / 1-cycle-per-row bf16 rate.
The matmul path runs in bf16 (weights, x, hm), which (a) halves HBM
traffic vs f32 to ~286 MB/core (~0.95 ms at ~300 GB/s, fully hidden
under compute -- the f32 version was DMA-bound at 2.14 ms), and
(b) enables fast-weight-load so LDWEIGHTS hides under the matmuls.
PSUM accumulation and the y accumulator stay f32.

I=11008 is processed exactly (no padding) as 21 i-chunks of 512 plus one
tail chunk of 256, grouped in PAIRS: gate/up for chunk A, gate/up for
chunk B, then one combined down pass whose PSUM groups accumulate over
both chunks' hm (8-12 matmuls per PSUM bank before the single DVE
y-accumulate).  Pairing keeps the PE ahead of the DVE on PSUM-bank
recycling (a 2-matmul tail group is faster than its 0.7us DVE drain),
halves the y-accumulate traffic, and halves the chunk-boundary count.

Device-side layouts (all transposes/tiling done on HOST in numpy so
every device DMA is a plain contiguous partition-major copy):
  x_host  [128, HS=32, T]          x^T tiled: [p, hs, t] = x[t, hs*128+p]
  wg main [21, 16, 128, 2, 512]    Wg^T tiled; tail [16, 128, 2, 256]
  wu      same
  wd main [21, 8, 128, 4, 512]     Wd^T tiled; tail [8, 128, 2, 512]
  y out   [TT=4, 128, H]           y[tt*128+p, o]  (natural token-major)

Per-core kernel (per i-chunk of W columns, nsub = W/128):
  gate/up:  psum_g/u[it][128i, T] += Wg^T[h,i].T @ x^T[h,t]   (32 h-subtiles,
            g/u interleaved so ACT+DVE drain pipelines into the last MMs)
  mid:      hm[it] = silu(psum_g) * psum_u          (ACT + DVE, hm in bf16)
  down:     psum_y[128t, 512o]    += hm[is][:,tt].T-as-lhsT @ Wd^T[i,o]
            y_sbuf[tt] += psum_y                    (DVE accumulate, f32)
"""

import numpy as np
import ml_dtypes

import concourse.bass as bass
import concourse.mybir as mybir
import concourse.tile as tile
from concourse import bacc
from concourse.bass_utils import run_bass_kernel_spmd

F32 = mybir.dt.float32
BF16 = mybir.dt.bfloat16
P = 128
ICW = 512  # main i-chunk width (4 subtiles of 128)
OCW = 512  # o-chunk width

# full-size problem constants
B, S, H, I = 2, 2048, 4096, 11008
NCORES = 8
T = (B * S) // NCORES  # 512 tokens per core

NP_BF16 = ml_dtypes.bfloat16


def make_chunks(Iin):
    """Split Iin into chunks of ICW plus at most one smaller tail chunk."""
    assert Iin % P == 0
    n_main = Iin // ICW
    chunks = [ICW] * n_main
    if Iin % ICW:
        chunks.append(Iin % ICW)
    return chunks


def build_nc(T, H, Iin, wg_bufs=10, wd_bufs=6, hm_bufs=12, sg_bufs=4, mm_dt=BF16,
             use_silu=True):
    assert T % P == 0 and T <= 512
    assert H % 512 == 0 and (H // P) % 2 == 0
    HS = H // P       # h subtiles (contraction for gate/up)
    NO = H // OCW     # o chunks
    TT = T // P       # token tiles
    chunks = make_chunks(Iin)
    n_main = sum(1 for w in chunks if w == ICW)
    w_tail = chunks[-1] if chunks[-1] != ICW else 0
    # pair up chunks for the combined down pass
    pairs = [list(range(i, min(i + 2, len(chunks)))) for i in range(0, len(chunks), 2)]

    nc = bacc.Bacc("TRN2", target_bir_lowering=False, debug=False)
    x_d = nc.dram_tensor("x", [P, HS, T], mm_dt, kind="ExternalInput").ap()
    wg_d = nc.dram_tensor("wg", [n_main, HS // 2, P, 2, ICW], mm_dt, kind="ExternalInput").ap()
    wu_d = nc.dram_tensor("wu", [n_main, HS // 2, P, 2, ICW], mm_dt, kind="ExternalInput").ap()
    wd_d = nc.dram_tensor("wd", [n_main, NO, P, ICW // P, OCW], mm_dt, kind="ExternalInput").ap()
    if w_tail:
        wgt_d = nc.dram_tensor("wgt", [HS // 2, P, 2, w_tail], mm_dt, kind="ExternalInput").ap()
        wut_d = nc.dram_tensor("wut", [HS // 2, P, 2, w_tail], mm_dt, kind="ExternalInput").ap()
        wdt_d = nc.dram_tensor("wdt", [NO, P, w_tail // P, OCW], mm_dt, kind="ExternalInput").ap()
    y_d = nc.dram_tensor("y", [TT, P, H], F32, kind="ExternalOutput").ap()

    with tile.TileContext(nc) as tc:
        with (
            tc.tile_pool(name="xp", bufs=1) as xp,
            tc.tile_pool(name="yp", bufs=1) as yp,
            tc.tile_pool(name="wgp", bufs=wg_bufs) as wgp,
            tc.tile_pool(name="wup", bufs=wg_bufs) as wup,
            tc.tile_pool(name="wdp", bufs=wd_bufs) as wdp,
            tc.tile_pool(name="hmp", bufs=hm_bufs) as hmp,
            tc.tile_pool(name="sgp", bufs=sg_bufs) as sgp,
            tc.tile_pool(name="ps", bufs=8, space="PSUM") as ps,
        ):
            # resident x^T (4 MiB bf16) in 4 independent tiles so the first
            # matmuls only wait on the first quarter; the first chunk's j=0
            # weight tiles are issued right after x quarter 0
            if HS >= 32:
                x_widths = [4, 4, 8, HS - 16]
            else:
                x_widths = [HS]
            xts, xmap, off = [], [], 0
            for q, w in enumerate(x_widths):
                xt_ = xp.tile([P, w, T], mm_dt, tag=f"x{q}", name=f"x{q}")
                xts.append((xt_, x_d[:, off:off + w, :]))
                xmap += [(q, k) for k in range(w)]
                off += w
            # first x slice + first weight tiles win the early DMA window;
            # the remaining x slices are issued interleaved with the first
            # chunk's j-loop weight DMAs (pending_x below)
            nc.sync.dma_start(out=xts[0][0], in_=xts[0][1])
            pre_gt = wgp.tile([P, 2, chunks[0]], mm_dt, tag="wg")
            nc.sync.dma_start(
                out=pre_gt, in_=wgt_d[0] if (w_tail and len(chunks) == 1) else wg_d[0, 0])
            pre_ut = wup.tile([P, 2, chunks[0]], mm_dt, tag="wu")
            nc.sync.dma_start(
                out=pre_ut, in_=wut_d[0] if (w_tail and len(chunks) == 1) else wu_d[0, 0])
            pending_x = list(xts[1:])

            def xsl(hs):
                q, k = xmap[hs]
                return xts[q][0][:, k, :]

            # warm up the PE HAM clock gate during the initial DMA wait:
            # a few matmuls on zeroed SBUF keep the PE busy from ~7us so the
            # 4096-cycle activity window has already un-throttled the clock
            # when the first real matmul issues
            warm = hmp.tile([P, T], mm_dt, tag="hm", name="warm")
            nc.vector.memset(warm, 0.0)
            pwarm = ps.tile([P, T], F32, tag="ps", name="pwarm")
            for _ in range(8):
                nc.tensor.matmul(pwarm, warm[:, 0:P], warm, start=True, stop=True)
            # resident y accumulator (8 MiB f32); the first down pass writes
            # it with tensor_copy so no memset is needed
            yt = [yp.tile([P, H], F32, name=f"y{tt}", tag=f"y{tt}") for tt in range(TT)]

            for ip, pair in enumerate(pairs):
                first_pair = ip == 0
                last_pair = ip == len(pairs) - 1
                hm_all = []  # [(hms, ic)] for this pair
                for ic in pair:
                    W = chunks[ic]
                    nsub = W // P
                    is_tail = w_tail and ic == len(chunks) - 1
                    # ---- gate/up projections, accumulated over all h ----
                    psg = [ps.tile([P, T], F32, tag="ps", name=f"psg{k}") for k in range(nsub)]
                    psu = [ps.tile([P, T], F32, tag="ps", name=f"psu{k}") for k in range(nsub)]
                    for j in range(HS // 2):
                        if ic == 0 and j == 0:
                            gt, ut = pre_gt, pre_ut
                        else:
                            gt = wgp.tile([P, 2, W], mm_dt, tag="wg")
                            nc.sync.dma_start(out=gt, in_=wgt_d[j] if is_tail else wg_d[ic, j])
                            ut = wup.tile([P, 2, W], mm_dt, tag="wu")
                            nc.sync.dma_start(out=ut, in_=wut_d[j] if is_tail else wu_d[ic, j])
                        if pending_x:
                            xt_, src = pending_x.pop(0)
                            nc.sync.dma_start(out=xt_, in_=src)
                        for h2 in range(2):
                            hs = 2 * j + h2
                            first, last = hs == 0, hs == HS - 1
                            for it in range(nsub):
                                nc.tensor.matmul(
                                    psg[it],
                                    gt[:, h2, it * P:(it + 1) * P],
                                    xsl(hs),
                                    start=first, stop=last,
                                )
                                nc.tensor.matmul(
                                    psu[it],
                                    ut[:, h2, it * P:(it + 1) * P],
                                    xsl(hs),
                                    start=first, stop=last,
                                )
                    # ---- silu(gate) * up -> hm tiles [i128, T] in bf16 ----
                    hms = []
                    for it in range(nsub):
                        sg = sgp.tile([P, T], F32, tag="sg")
                        if use_silu:
                            # native HW silu: one ACT op frees psg immediately
                            nc.scalar.activation(
                                sg, psg[it], mybir.ActivationFunctionType.Silu
                            )
                        else:
                            # CoreSim lacks Silu: sigmoid + extra DVE mul
                            nc.scalar.activation(
                                sg, psg[it], mybir.ActivationFunctionType.Sigmoid
                            )
                            nc.vector.tensor_mul(sg, sg, psg[it])
                        hm = hmp.tile([P, T], mm_dt, tag="hm")
                        nc.vector.tensor_mul(hm, sg, psu[it])
                        hms.append(hm)
                    hm_all.append((hms, ic))
                # ---- combined down projection for this pair of i-chunks ----
                for osc in range(NO):
                    mm_ops = []  # (hm tile, wd lhsT slice) per 128-wide i-subtile
                    for hms, ic in hm_all:
                        nsub = len(hms)
                        is_tail = w_tail and ic == len(chunks) - 1
                        wdt = wdp.tile([P, nsub, OCW], mm_dt, tag="wd")
                        nc.sync.dma_start(out=wdt, in_=wdt_d[osc] if is_tail else wd_d[ic, osc])
                        for s in range(nsub):
                            mm_ops.append((hms[s], wdt[:, s, :]))
                    osl = slice(osc * OCW, (osc + 1) * OCW)
                    for tt in range(TT):
                        py = ps.tile([P, OCW], F32, tag="ps", name="py")
                        for q, (hm, wsl) in enumerate(mm_ops):
                            nc.tensor.matmul(
                                py,
                                hm[:, tt * P:(tt + 1) * P],
                                wsl,
                                start=(q == 0), stop=(q == len(mm_ops) - 1),
                            )
                        if first_pair:
                            nc.vector.tensor_copy(yt[tt][:, osl], py)
                        else:
                            nc.vector.tensor_add(yt[tt][:, osl], yt[tt][:, osl], py)
                        if last_pair:
                            # y[:, osl] is final -- stream it out now
                            nc.sync.dma_start(out=y_d[tt][:, osl], in_=yt[tt][:, osl])

    nc.compile()
    return nc


def prep_weights(Wg, Wu, Wd, Iin):
    """Host-side re-tiling of the weights into the device DMA layouts."""
    _, Hh = Wg.shape
    HS = Hh // P
    NO = Hh // OCW
    chunks = make_chunks(Iin)
    n_main = sum(1 for w in chunks if w == ICW)
    w_tail = chunks[-1] if chunks[-1] != ICW else 0
    Im = n_main * ICW

    def gu_tile(Wx, off, w):
        # [j, p, h2, ii] = Wx[off + ii, (2j+h2)*128 + p]
        return Wx[off:off + w].reshape(w, HS // 2, 2, P).transpose(1, 3, 2, 0)

    def d_tile(Wx, off, w):
        # [osc, p, isub, oo] = Wx[osc*OCW + oo, off + isub*128 + p]
        return Wx[:, off:off + w].reshape(NO, OCW, w // P, P).transpose(0, 3, 2, 1)

    out = {
        "wg": np.ascontiguousarray(
            Wg[:Im].reshape(n_main, ICW, HS // 2, 2, P).transpose(0, 2, 4, 3, 1)
        ).astype(NP_BF16),
        "wu": np.ascontiguousarray(
            Wu[:Im].reshape(n_main, ICW, HS // 2, 2, P).transpose(0, 2, 4, 3, 1)
        ).astype(NP_BF16),
        "wd": np.ascontiguousarray(
            Wd[:, :Im].reshape(NO, OCW, n_main, ICW // P, P).transpose(2, 0, 4, 3, 1)
        ).astype(NP_BF16),
    }
    if w_tail:
        out["wgt"] = np.ascontiguousarray(gu_tile(Wg, Im, w_tail)).astype(NP_BF16)
        out["wut"] = np.ascontiguousarray(gu_tile(Wu, Im, w_tail)).astype(NP_BF16)
        out["wdt"] = np.ascontiguousarray(d_tile(Wd, Im, w_tail)).astype(NP_BF16)
    return out


def prep_x_shard(x2, c, T):
    """x2 [tokens, H] -> core c's [128, HS, T] tile layout (bf16)."""
    Hh = x2.shape[1]
    xs = x2[c * T:(c + 1) * T]  # [T, H]
    return np.ascontiguousarray(
        xs.reshape(T, Hh // P, P).transpose(2, 1, 0)
    ).astype(NP_BF16)


def run_on_cores(nc, in_maps, **kwargs):
    return run_bass_kernel_spmd(nc, in_maps, core_ids=list(range(len(in_maps))), **kwargs)


_NC_CACHE = {}


def _get_nc():
    key = (T, H, I)
    if key not in _NC_CACHE:
        _NC_CACHE[key] = build_nc(T, H, I)
    return _NC_CACHE[key]


def kernel(x, Wg, Wu, Wd, _trace=False, _trace_kwargs=None):
    x = np.asarray(x, np.float32)
    Wg = np.asarray(Wg, np.float32)
    Wu = np.asarray(Wu, np.float32)
    Wd = np.asarray(Wd, np.float32)

    nc = _get_nc()
    w_maps = prep_weights(Wg, Wu, Wd, I)
    x2 = x.reshape(B * S, H)
    in_maps = [{"x": prep_x_shard(x2, c, T), **w_maps} for c in range(NCORES)]
    kwargs = {}
    if _trace:
        kwargs["trace"] = True
        kwargs.update(_trace_kwargs or {})
    res = run_on_cores(nc, in_maps, **kwargs)
    shards = [res.results[c]["y"].reshape(T, H) for c in range(NCORES)]
    y = np.concatenate(shards, axis=0).reshape(B, S, H)
    if _trace:
        return y, res
    return y


# revision 11
# speedup vs baseline: 1.1837x; 1.0001x over previous
"""Fused SwiGLU MLP (gate/up/down) Trainium2 Bass kernel.

Problem: y = down( silu(x @ Wg^T) * (x @ Wu^T) ) with
  x  [B=2, S=2048, H=4096]  f32
  Wg [I=11008, H]           f32   (gate proj, [out,in])
  Wu [I=11008, H]           f32
  Wd [H, I]                 f32

Strategy: data-parallel over tokens across the 8 NeuronCores.
Each core gets T = 4096/8 = 512 tokens and the full (replicated) weights,
computing the entire MLP for its token shard.  No collectives; the host
just concatenates the 8 token shards.

Per-core work: 138.6 GFLOP -> 8256 matmuls of [128k x 128m] @ [128 x 512]
= 4.23M PE cycles = 1.76 ms at the 2.4 GHz / 1-cycle-per-row bf16 rate.
The matmul path runs in bf16 (weights, x, hm), which (a) halves HBM
traffic vs f32 to ~286 MB/core (~0.95 ms at ~300 GB/s, fully hidden
under compute -- the f32 version was DMA-bound at 2.14 ms), and
(b) enables fast-weight-load so LDWEIGHTS hides under the matmuls.
PSUM accumulation and the y accumulator stay f32.

I=11008 is processed exactly (no padding) as 21 i-chunks of 512 plus one
tail chunk of 256, grouped in PAIRS: gate/up for chunk A, gate/up for
chunk B, then one combined down pass whose PSUM groups accumulate over
both chunks' hm (8-12 matmuls per PSUM bank before the single DVE
y-accumulate).  Pairing keeps the PE ahead of the DVE on PSUM-bank
recycling (a 2-matmul tail group is faster than its 0.7us DVE drain),
halves the y-accumulate traffic, and halves the chunk-boundary count.

Device-side layouts (all transposes/tiling done on HOST in numpy so
every device DMA is a plain contiguous partition-major copy):
  x_host  [128, HS=32, T]          x^T tiled: [p, hs, t] = x[t, hs*128+p]
  wg main [21, 16, 128, 2, 512]    Wg^T tiled; tail [16, 128, 2, 256]
  wu      same
  wd main [21, 8, 128, 4, 512]     Wd^T tiled; tail [8, 128, 2, 512]
  y out   [TT=4, 128, H]           y[tt*128+p, o]  (natural token-major)

Per-core kernel (per i-chunk of W columns, nsub = W/128):
  gate/up:  psum_g/u[it][128i, T] += Wg^T[h,i].T @ x^T[h,t]   (32 h-subtiles,
            g/u interleaved so ACT+DVE drain pipelines into the last MMs)
  mid:      hm[it] = silu(psum_g) * psum_u          (ACT + DVE, hm in bf16)
  down:     psum_y[128t, 512o]    += hm[is][:,tt].T-as-lhsT @ Wd^T[i,o]
            y_sbuf[tt] += psum_y                    (DVE accumulate, f32)
"""

import numpy as np
import ml_dtypes

import concourse.bass as bass
import concourse.mybir as mybir
import concourse.tile as tile
from concourse import bacc
from concourse.bass_utils import run_bass_kernel_spmd

F32 = mybir.dt.float32
BF16 = mybir.dt.bfloat16
P = 128
ICW = 512  # main i-chunk width (4 subtiles of 128)
OCW = 512  # o-chunk width

# full-size problem constants
B, S, H, I = 2, 2048, 4096, 11008
NCORES = 8
T = (B * S) // NCORES  # 512 tokens per core

NP_BF16 = ml_dtypes.bfloat16


def make_chunks(Iin):
    """Split Iin into chunks of ICW plus at most one smaller tail chunk."""
    assert Iin % P == 0
    n_main = Iin // ICW
    chunks = [ICW] * n_main
    if Iin % ICW:
        chunks.append(Iin % ICW)
    return chunks


def build_nc(T, H, Iin, wg_bufs=10, wd_bufs=8, hm_bufs=16, sg_bufs=4, mm_dt=BF16,
             use_silu=True):
    assert T % P == 0 and T <= 512
    assert H % 512 == 0 and (H // P) % 2 == 0
    HS = H // P       # h subtiles (contraction for gate/up)
    NO = H // OCW     # o chunks
    TT = T // P       # token tiles
    chunks = make_chunks(Iin)
    n_main = sum(1 for w in chunks if w == ICW)
    w_tail = chunks[-1] if chunks[-1] != ICW else 0
    # pair up chunks for the combined down pass; merge the final two
    # pairs into one group of up to 4 so the last down phase has enough
    # PE work to hide its wd-tile DMAs plus the streamed y writeout
    pairs = [list(range(i, min(i + 2, len(chunks)))) for i in range(0, len(chunks), 2)]
    if len(pairs) >= 2 and len(pairs[-1]) + len(pairs[-2]) <= 4:
        pairs[-2].extend(pairs.pop())

    nc = bacc.Bacc("TRN2", target_bir_lowering=False, debug=False)
    x_d = nc.dram_tensor("x", [P, HS, T], mm_dt, kind="ExternalInput").ap()
    wg_d = nc.dram_tensor("wg", [n_main, HS // 2, P, 2, ICW], mm_dt, kind="ExternalInput").ap()
    wu_d = nc.dram_tensor("wu", [n_main, HS // 2, P, 2, ICW], mm_dt, kind="ExternalInput").ap()
    wd_d = nc.dram_tensor("wd", [n_main, NO, P, ICW // P, OCW], mm_dt, kind="ExternalInput").ap()
    if w_tail:
        wgt_d = nc.dram_tensor("wgt", [HS // 2, P, 2, w_tail], mm_dt, kind="ExternalInput").ap()
        wut_d = nc.dram_tensor("wut", [HS // 2, P, 2, w_tail], mm_dt, kind="ExternalInput").ap()
        wdt_d = nc.dram_tensor("wdt", [NO, P, w_tail // P, OCW], mm_dt, kind="ExternalInput").ap()
    y_d = nc.dram_tensor("y", [TT, P, H], F32, kind="ExternalOutput").ap()

    with tile.TileContext(nc) as tc:
        with (
            tc.tile_pool(name="xp", bufs=1) as xp,
            tc.tile_pool(name="yp", bufs=1) as yp,
            tc.tile_pool(name="wgp", bufs=wg_bufs) as wgp,
            tc.tile_pool(name="wup", bufs=wg_bufs) as wup,
            tc.tile_pool(name="wdp", bufs=wd_bufs) as wdp,
            tc.tile_pool(name="hmp", bufs=hm_bufs) as hmp,
            tc.tile_pool(name="sgp", bufs=sg_bufs) as sgp,
            tc.tile_pool(name="ps", bufs=8, space="PSUM") as ps,
        ):
            # resident x^T (4 MiB bf16) in 4 independent tiles so the first
            # matmuls only wait on the first quarter; the first chunk's j=0
            # weight tiles are issued right after x quarter 0
            if HS >= 32:
                x_widths = [4, 4, 8, HS - 16]
            else:
                x_widths = [HS]
            xts, xmap, off = [], [], 0
            for q, w in enumerate(x_widths):
                xt_ = xp.tile([P, w, T], mm_dt, tag=f"x{q}", name=f"x{q}")
                xts.append((xt_, x_d[:, off:off + w, :]))
                xmap += [(q, k) for k in range(w)]
                off += w
            # first x slice + first weight tiles win the early DMA window;
            # the remaining x slices are issued interleaved with the first
            # chunk's j-loop weight DMAs (pending_x below)
            nc.sync.dma_start(out=xts[0][0], in_=xts[0][1])
            pre_gt = wgp.tile([P, 2, chunks[0]], mm_dt, tag="wg")
            nc.sync.dma_start(
                out=pre_gt, in_=wgt_d[0] if (w_tail and len(chunks) == 1) else wg_d[0, 0])
            pre_ut = wup.tile([P, 2, chunks[0]], mm_dt, tag="wu")
            nc.sync.dma_start(
                out=pre_ut, in_=wut_d[0] if (w_tail and len(chunks) == 1) else wu_d[0, 0])
            pending_x = list(xts[1:])

            def xsl(hs):
                q, k = xmap[hs]
                return xts[q][0][:, k, :]

            # warm up the PE HAM clock gate during the initial DMA wait:
            # a few matmuls on zeroed SBUF keep the PE busy from ~7us so the
            # 4096-cycle activity window has already un-throttled the clock
            # when the first real matmul issues
            warm = hmp.tile([P, T], mm_dt, tag="hm", name="warm")
            nc.vector.memset(warm, 0.0)
            pwarm = ps.tile([P, T], F32, tag="ps", name="pwarm")
            for _ in range(8):
                nc.tensor.matmul(pwarm, warm[:, 0:P], warm, start=True, stop=True)
            # resident y accumulator (8 MiB f32); the first down pass writes
            # it with tensor_copy so no memset is needed
            yt = [yp.tile([P, H], F32, name=f"y{tt}", tag=f"y{tt}") for tt in range(TT)]

            for ip, pair in enumerate(pairs):
                first_pair = ip == 0
                last_pair = ip == len(pairs) - 1
                hm_all = []  # [(hms, ic)] for this pair
                for ic in pair:
                    W = chunks[ic]
                    nsub = W // P
                    is_tail = w_tail and ic == len(chunks) - 1
                    # ---- gate/up projections, accumulated over all h ----
                    psg = [ps.tile([P, T], F32, tag="ps", name=f"psg{k}") for k in range(nsub)]
                    psu = [ps.tile([P, T], F32, tag="ps", name=f"psu{k}") for k in range(nsub)]
                    for j in range(HS // 2):
                        if ic == 0 and j == 0:
                            gt, ut = pre_gt, pre_ut
                        else:
                            gt = wgp.tile([P, 2, W], mm_dt, tag="wg")
                            nc.sync.dma_start(out=gt, in_=wgt_d[j] if is_tail else wg_d[ic, j])
                            ut = wup.tile([P, 2, W], mm_dt, tag="wu")
                            nc.sync.dma_start(out=ut, in_=wut_d[j] if is_tail else wu_d[ic, j])
                        if pending_x:
                            xt_, src = pending_x.pop(0)
                            nc.sync.dma_start(out=xt_, in_=src)
                        for h2 in range(2):
                            hs = 2 * j + h2
                            first, last = hs == 0, hs == HS - 1
                            for it in range(nsub):
                                nc.tensor.matmul(
                                    psg[it],
                                    gt[:, h2, it * P:(it + 1) * P],
                                    xsl(hs),
                                    start=first, stop=last,
                                )
                                nc.tensor.matmul(
                                    psu[it],
                                    ut[:, h2, it * P:(it + 1) * P],
                                    xsl(hs),
                                    start=first, stop=last,
                                )
                    # ---- silu(gate) * up -> hm tiles [i128, T] in bf16 ----
                    hms = []
                    for it in range(nsub):
                        sg = sgp.tile([P, T], F32, tag="sg")
                        if use_silu:
                            # native HW silu: one ACT op frees psg immediately
                            nc.scalar.activation(
                                sg, psg[it], mybir.ActivationFunctionType.Silu
                            )
                        else:
                            # CoreSim lacks Silu: sigmoid + extra DVE mul
                            nc.scalar.activation(
                                sg, psg[it], mybir.ActivationFunctionType.Sigmoid
                            )
                            nc.vector.tensor_mul(sg, sg, psg[it])
                        hm = hmp.tile([P, T], mm_dt, tag="hm")
                        nc.vector.tensor_mul(hm, sg, psu[it])
                        hms.append(hm)
                    hm_all.append((hms, ic))
                # ---- combined down projection for this pair of i-chunks ----
                for osc in range(NO):
                    mm_ops = []  # (hm tile, wd lhsT slice) per 128-wide i-subtile
                    for hms, ic in hm_all:
                        nsub = len(hms)
                        is_tail = w_tail and ic == len(chunks) - 1
                        wdt = wdp.tile([P, nsub, OCW], mm_dt, tag="wd")
                        nc.sync.dma_start(out=wdt, in_=wdt_d[osc] if is_tail else wd_d[ic, osc])
                        for s in range(nsub):
                            mm_ops.append((hms[s], wdt[:, s, :]))
                    osl = slice(osc * OCW, (osc + 1) * OCW)
                    for tt in range(TT):
                        py = ps.tile([P, OCW], F32, tag="ps", name="py")
                        for q, (hm, wsl) in enumerate(mm_ops):
                            nc.tensor.matmul(
                                py,
                                hm[:, tt * P:(tt + 1) * P],
                                wsl,
                                start=(q == 0), stop=(q == len(mm_ops) - 1),
                            )
                        if first_pair:
                            nc.vector.tensor_copy(yt[tt][:, osl], py)
                        else:
                            nc.vector.tensor_add(yt[tt][:, osl], yt[tt][:, osl], py)
                        if last_pair:
                            # y[:, osl] is final -- stream it out now
                            nc.sync.dma_start(out=y_d[tt][:, osl], in_=yt[tt][:, osl])

    nc.compile()
    return nc


def prep_weights(Wg, Wu, Wd, Iin):
    """Host-side re-tiling of the weights into the device DMA layouts."""
    _, Hh = Wg.shape
    HS = Hh // P
    NO = Hh // OCW
    chunks = make_chunks(Iin)
    n_main = sum(1 for w in chunks if w == ICW)
    w_tail = chunks[-1] if chunks[-1] != ICW else 0
    Im = n_main * ICW

    def gu_tile(Wx, off, w):
        # [j, p, h2, ii] = Wx[off + ii, (2j+h2)*128 + p]
        return Wx[off:off + w].reshape(w, HS // 2, 2, P).transpose(1, 3, 2, 0)

    def d_tile(Wx, off, w):
        # [osc, p, isub, oo] = Wx[osc*OCW + oo, off + isub*128 + p]
        return Wx[:, off:off + w].reshape(NO, OCW, w // P, P).transpose(0, 3, 2, 1)

    out = {
        "wg": np.ascontiguousarray(
            Wg[:Im].reshape(n_main, ICW, HS // 2, 2, P).transpose(0, 2, 4, 3, 1)
        ).astype(NP_BF16),
        "wu": np.ascontiguousarray(
            Wu[:Im].reshape(n_main, ICW, HS // 2, 2, P).transpose(0, 2, 4, 3, 1)
        ).astype(NP_BF16),
        "wd": np.ascontiguousarray(
            Wd[:, :Im].reshape(NO, OCW, n_main, ICW // P, P).transpose(2, 0, 4, 3, 1)
        ).astype(NP_BF16),
    }
    if w_tail:
        out["wgt"] = np.ascontiguousarray(gu_tile(Wg, Im, w_tail)).astype(NP_BF16)
        out["wut"] = np.ascontiguousarray(gu_tile(Wu, Im, w_tail)).astype(NP_BF16)
        out["wdt"] = np.ascontiguousarray(d_tile(Wd, Im, w_tail)).astype(NP_BF16)
    return out


def prep_x_shard(x2, c, T):
    """x2 [tokens, H] -> core c's [128, HS, T] tile layout (bf16)."""
    Hh = x2.shape[1]
    xs = x2[c * T:(c + 1) * T]  # [T, H]
    return np.ascontiguousarray(
        xs.reshape(T, Hh // P, P).transpose(2, 1, 0)
    ).astype(NP_BF16)


def run_on_cores(nc, in_maps, **kwargs):
    return run_bass_kernel_spmd(nc, in_maps, core_ids=list(range(len(in_maps))), **kwargs)


_NC_CACHE = {}


def _get_nc():
    key = (T, H, I)
    if key not in _NC_CACHE:
        _NC_CACHE[key] = build_nc(T, H, I)
    return _NC_CACHE[key]


def kernel(x, Wg, Wu, Wd, _trace=False, _trace_kwargs=None):
    x = np.asarray(x, np.float32)
    Wg = np.asarray(Wg, np.float32)
    Wu = np.asarray(Wu, np.float32)
    Wd = np.asarray(Wd, np.float32)

    nc = _get_nc()
    w_maps = prep_weights(Wg, Wu, Wd, I)
    x2 = x.reshape(B * S, H)
    in_maps = [{"x": prep_x_shard(x2, c, T), **w_maps} for c in range(NCORES)]
    kwargs = {}
    if _trace:
        kwargs["trace"] = True
        kwargs.update(_trace_kwargs or {})
    res = run_on_cores(nc, in_maps, **kwargs)
    shards = [res.results[c]["y"].reshape(T, H) for c in range(NCORES)]
    y = np.concatenate(shards, axis=0).reshape(B, S, H)
    if _trace:
        return y, res
    return y
